# revision 35
# baseline (speedup 1.0000x reference)
"""Masked self-attention Trainium2 kernel (v8 — queue-balanced, PE-summed tail).

Reference computes (per batch b):
    key   = x @ Wk.T            [S, 64]
    query = x @ Wq.T            [S, 64]
    value = x @ Wv.T            [S, 128]
    kT_m  = tril(key.T)         [64, S]   -- element (d, s) kept iff s <= d
    out   = softmax(query @ kT_m, axis=-1) @ value

tril zeroes every score column s >= 64, so with fixed shift c:

    out[t] = (sum_{s<64} e^{z_st-c} v[s] + e^{-c} Vtail) /
             (sum_{s<64} e^{z_st-c}      + e^{-c} (S-64))

with Vtail = (sum_{s>=64} x[s]) @ Wv.T.  Per core (batch b, half h):
z = Wz.T @ xow with Wz = tril-masked key64 folded into Wq (65th row = 0 so
pT row 64 = e^{-c}); out tiles are single K=65 matmuls against
vaug = [v64 rows | vtail row], den in the 129th column.

Trace-driven design notes (measured on HW, ~25us of which ~8.5us is the
fixed walrus sem-clear epilogue and ~2.5us fixed DMA startup):
- Queues (~150 GB/s per HWDGE queue, ~100 GB/s gpsimd SWDGE, ~0.9us
  DMA-completion-sem latency, ~0.7us per-queue DMA transition): sync:
  wpk -> xowb1; scalar: xowa -> xowb2; gpsimd: xo8a -> xo8b.  The own
  half is split 3 ways so z chunks and the own-half reduces unblock as
  their bytes land.
- Other half ships as fp8 natural-TILED 128-token blocks; its column sum
  runs on the PE as 16 tiny (block x fp8-ones) matmuls accumulating one
  PSUM column, so the scalar engine runs only the 4 exps plus scales.
- The tile scheduler orders each engine stream by ITS OWN sim; data-ready
  order on HW differs.  tile_wait_until hints (in sim-ms) pin the PE
  stream to [preamble, z0..z3, fp8 sums, vtail, tiles]; without them the
  sums (late fp8 data) park ahead of data-ready z matmuls and stall the
  in-order PE for microseconds.
- PSUM banks (8 x 2KB): z1/z2 ping-pong in zps; z0 + [osum | vtail |
  tile15] share one bank via sequential same-tag reuse; z3 takes the
  first oa-pool bank so its reuser (tiles 12-14) is exp3-gated anyway;
  tiles 0-14 pack 3-per-bank into the remaining 5 banks.  A PSUM bank
  tolerates only ONE open (start=True..stop) accumulation at a time --
  opening split-K accumulations in sibling slots corrupts them.
- Normalize: one strided [128,3] reciprocal per bank; 16 per-tile scales
  (PSUM->SBUF bf16, ~0.35/0.45us) split lane-pure across DVE and ACT so
  a store group never waits the other engine; stores alternate the sync
  and gpsimd queues.
"""

import numpy as np

import concourse.bass as bass
import concourse.bacc as bacc
import concourse.tile as tile
from concourse import mybir
from concourse.bass_utils import run_bass_kernel_spmd

F32 = mybir.dt.float32
F16 = mybir.dt.float16
BF16 = mybir.dt.bfloat16
FP8 = mybir.dt.float8e4
AF = mybir.ActivationFunctionType
AX = mybir.AxisListType
ALU = mybir.AluOpType

B, S, E, KD = 4, 4096, 128, 64
HALF = S // 2            # tokens handled per core
NCORES = 8
CHUNK = 512              # tokens per z-matmul / exp
NCHUNK = HALF // CHUNK
TSUB = 128               # tokens per output tile
NTILE = HALF // TSUB     # 16
CSHIFT = 20.0            # fixed softmax shift
NTAIL = float(S - KD)    # 4032 all-zero score columns
W = E + 1                # 129: num cols + den col per tile
NBANK = 6                # 3 tiles per PSUM bank (last bank holds 1)

# wpk columns: [x64T(64) | WkT(64) | Wq(128, rows 0:64) | tri(64, rows 0:64)]
X64_OFF, WK_OFF, WV_OFF, WQ_OFF, TRI_OFF = 0, KD, 2 * KD, 2 * KD + E, 2 * KD + 2 * E
WPK_COLS = 2 * KD + 2 * E + KD  # 448


def _build_nc() -> bass.Bass:
    nc = bacc.Bacc("TRN2", target_bir_lowering=False, debug=False)

    wpk = nc.dram_tensor("wpk", [E, WPK_COLS], F16, kind="ExternalInput").ap()
    xowa = nc.dram_tensor("xowa", [E, 1024], F16, kind="ExternalInput").ap()
    xowb1 = nc.dram_tensor("xowb1", [E, 512], F16, kind="ExternalInput").ap()
    xowb2 = nc.dram_tensor("xowb2", [E, 512], F16, kind="ExternalInput").ap()
    xo8a = nc.dram_tensor("xo8a", [E, 1024], FP8, kind="ExternalInput").ap()
    xo8b = nc.dram_tensor("xo8b", [E, 1024], FP8, kind="ExternalInput").ap()
    outs = [
        nc.dram_tensor(f"o{g}", [TSUB, 4, E], BF16, kind="ExternalOutput").ap()
        for g in range(4)
    ]

    with tile.TileContext(nc) as tc:
        with (
            tc.tile_pool(name="singles", bufs=1) as singles,
            tc.tile_pool(name="zps", bufs=2, space="PSUM") as zps,
            tc.tile_pool(name="misc_ps", bufs=1, space="PSUM") as misc_ps,
            tc.tile_pool(name="oa_ps", bufs=5, space="PSUM") as oa_ps,
            tc.tile_pool(name="recs", bufs=6) as recs,
            tc.tile_pool(name="obs", bufs=4) as obs,
        ):
            # ---- DMA in (queue order == issue order per engine) ----
            wpk_sb = singles.tile([E, WPK_COLS], F16)
            nc.sync.dma_start(wpk_sb[:], wpk)
            xow_sb = singles.tile([E, HALF], F16)
            xo8_sb = singles.tile([E, HALF], FP8)
            nc.sync.dma_start(xow_sb[:, 1024:1536], xowb1)
            nc.scalar.dma_start(xow_sb[:, 0:1024], xowa)
            nc.scalar.dma_start(xow_sb[:, 1536:2048], xowb2)
            nc.gpsimd.dma_start(xo8_sb[:, 0:1024], xo8a)
            nc.gpsimd.dma_start(xo8_sb[:, 1024:2048], xo8b)

            x64T_sb = wpk_sb[:, X64_OFF : X64_OFF + KD]
            wkT_sb = wpk_sb[:, WK_OFF : WK_OFF + KD]
            wvT_sb = wpk_sb[:, WV_OFF : WV_OFF + E]
            wq_sb = wpk_sb[0:KD, WQ_OFF : WQ_OFF + E]
            tri_sb = wpk_sb[0:KD, TRI_OFF : TRI_OFF + KD]

            # ---- constants (gpsimd is otherwise idle early) ----
            wzaug_sb = singles.tile([E, KD + 1], F16)
            nc.gpsimd.memset(wzaug_sb[:, KD : KD + 1], 0.0)
            vaug_sb = singles.tile([KD + 1, W], BF16)
            nc.gpsimd.memset(vaug_sb[0:KD, E : E + 1], 1.0)
            nc.gpsimd.memset(vaug_sb[KD : KD + 1, E : E + 1], NTAIL)
            nbias_sb = singles.tile([KD + 1, 1], F32)
            nc.gpsimd.memset(nbias_sb[:], -CSHIFT)
            ones8_sb = singles.tile([E, 1], FP8)
            nc.gpsimd.memset(ones8_sb[:], 1.0)

            # ---- preamble: build Wz (score weights) and v64 ----
            with tc.high_priority():
                kT_ps = zps.tile([KD, KD], F32, tag="z", name="kT_ps")
                nc.tensor.matmul(kT_ps[:], wkT_sb, x64T_sb, start=True, stop=True)
                kmT_sb = singles.tile([KD, KD], F16)
                nc.vector.tensor_mul(kmT_sb[:], kT_ps[:], tri_sb)
                wzT_ps = zps.tile([E, KD], F32, tag="z", name="wzT_ps")
                nc.tensor.matmul(wzT_ps[:], wq_sb, kmT_sb[:], start=True, stop=True)
                nc.vector.tensor_copy(wzaug_sb[:, 0:KD], wzT_ps[:])
                v64_ps = zps.tile([KD, E], F32, tag="z", name="v64_ps")
                nc.tensor.matmul(v64_ps[:], x64T_sb, wvT_sb, start=True, stop=True)
                nc.scalar.activation(vaug_sb[0:KD, 0:E], v64_ps[:], AF.Copy)

            x64s_sb = singles.tile([E, 1], F32)
            nc.vector.reduce_sum(out=x64s_sb[:], in_=x64T_sb, axis=AX.X)

            # ---- z chunks + exps; fp8 PE-sums woven into PE gaps ----
            pT_sb = singles.tile([KD + 1, HALF], BF16)

            def zexp(c, w):
                cs = slice(c * CHUNK, (c + 1) * CHUNK)
                if c == 0:
                    z_ps = misc_ps.tile([KD + 1, CHUNK], F32, tag="m", name="z0_ps")
                elif c == 3:
                    # fresh bank from the oa pool: its eventual reuser
                    # (tiles 12-14) is gated on exp3 anyway
                    z_ps = oa_ps.tile([KD + 1, CHUNK], F32, tag="oa", name="z3_ps")
                else:
                    z_ps = zps.tile([KD + 1, CHUNK], F32, tag="z", name=f"z{c}_ps")
                with tc.tile_wait_until(w):
                    nc.tensor.matmul(
                        z_ps[:], wzaug_sb[:], xow_sb[:, cs], start=True, stop=True
                    )
                    nc.scalar.activation(
                        pT_sb[0 : KD + 1, cs], z_ps[:], AF.Exp, bias=nbias_sb[:]
                    )

            def osums(src, half, first, last):
                for j in range(8):
                    js = slice(half * 1024 + j * TSUB, half * 1024 + (j + 1) * TSUB)
                    nc.tensor.matmul(
                        osum_ps, src[:, js], ones8_sb[:],
                        start=(first and j == 0), stop=(last and j == 7),
                    )

            zexp(0, 0.0005)
            zexp(1, 0.001)
            zexp(2, 0.0015)
            zexp(3, 0.002)
            # misc pack shares the z0 bank (sequential same-tag reuse):
            # col 0 = osum, row0 cols 1:130 = vtail, cols 130:259 = tile 15
            mpk = misc_ps.tile([E, 263], F32, tag="m", name="miscpack_ps")
            osum_ps = mpk[:, 0:1]
            with tc.tile_wait_until(0.005):
                osums(xo8_sb, 0, True, False)
                osums(xo8_sb, 1, False, True)

            # ---- own-half sums (DVE) and the tail vector ----
            rdA_sb = singles.tile([E, 1], F32)
            rdB_sb = singles.tile([E, 1], F32)
            rdC_sb = singles.tile([E, 1], F32)
            with tc.tile_wait_until(0.003):
                nc.vector.reduce_sum(out=rdA_sb[:], in_=xow_sb[:, 0:1024], axis=AX.X)
            with tc.tile_wait_until(0.004):
                nc.vector.reduce_sum(out=rdC_sb[:], in_=xow_sb[:, 1024:1536], axis=AX.X)
            with tc.tile_wait_until(0.0045):
                nc.vector.reduce_sum(out=rdB_sb[:], in_=xow_sb[:, 1536:2048], axis=AX.X)
            w_sb = singles.tile([E, 1], F32)
            nc.vector.tensor_sub(w_sb[:], x64s_sb[:], rdB_sb[:])
            u1_sb = singles.tile([E, 1], F32)
            nc.vector.scalar_tensor_tensor(
                u1_sb[:], rdA_sb[:], rdC_sb[:], osum_ps, ALU.add, ALU.add
            )
            tailh_sb = singles.tile([E, 1], F16)
            nc.vector.tensor_sub(tailh_sb[:], u1_sb[:], w_sb[:])

            vtail_ps = mpk[0:1, 1:129]
            nc.tensor.matmul(vtail_ps, tailh_sb[:], wvT_sb, start=True, stop=True)
            with tc.tile_wait_until(0.0055):
                nc.scalar.activation(vaug_sb[KD : KD + 1, 0:E], vtail_ps, AF.Copy)

            # ---- output tiles: 3 per PSUM bank, K=65 single matmuls ----
            # t0-14 fill the 5 oa banks exactly (3 per bank); t15 lives in
            # the misc pack (no 6th bank allocation / WAR stall)
            oa_banks = []
            slot_of = {}

            def slot_ap(t):
                if t == 15:
                    return mpk[:, 130 : 130 + W]
                b, j = slot_of[t]
                return oa_banks[b][:, j * W : j * W + W]

            for t in range(15):
                if t % 3 == 0:
                    oa_banks.append(
                        oa_ps.tile([TSUB, 3 * W], F32, tag="oa", name=f"oa{t // 3}")
                    )
                slot_of[t] = (t // 3, t % 3)
            for t in range(NTILE):
                ts = slice(t * TSUB, (t + 1) * TSUB)
                nc.tensor.matmul(
                    slot_ap(t), pT_sb[0 : KD + 1, ts], vaug_sb[:],
                    start=True, stop=True,
                )

            # ---- normalize + store ----
            rec_tiles = []
            for b in range(5):
                rec = recs.tile([TSUB, 3], F32, tag="rec", name=f"rec{b}")
                rec_tiles.append(rec)
                nc.vector.reciprocal(rec[:, 0:3], oa_banks[b][:, E :: W][:, 0:3])
            rec15 = recs.tile([TSUB, 1], F32, tag="rec15", name="rec15")
            nc.vector.reciprocal(rec15[:], mpk[:, 130 + E : 130 + E + 1])

            ob_tiles = [
                obs.tile([TSUB, 4, E], BF16, tag="ob", name=f"ob{g}")
                for g in range(4)
            ]
            ndone = [0] * 4
            for t in range(NTILE):
                g, gj = divmod(t, 4)
                ob = ob_tiles[g]
                oa = slot_ap(t)
                if t == 15:
                    rec = rec15[:]
                else:
                    b, j = slot_of[t]
                    rec = rec_tiles[b][:, j : j + 1]
                # lane-pure store groups: g0/g3 all-DVE, g1/g2 all-ACT
                if g in (0, 3):
                    nc.vector.tensor_scalar_mul(ob[:, gj, :], oa[:, 0:E], rec)
                else:
                    nc.scalar.activation(ob[:, gj, :], oa[:, 0:E], AF.Copy, scale=rec)
                ndone[g] += 1
                if ndone[g] == 4:
                    eng = (nc.sync, nc.gpsimd, nc.sync, nc.gpsimd)[g]
                    eng.dma_start(outs[g], ob[:])

    nc.compile()
    return nc


_NC_CACHE = None


def _get_nc() -> bass.Bass:
    global _NC_CACHE
    if _NC_CACHE is None:
        _NC_CACHE = _build_nc()
    return _NC_CACHE


def _make_in_maps(x, Wk, Wq, Wv):
    tri = (np.arange(KD)[:, None] >= np.arange(KD)[None, :]).astype(np.float16)
    wq_pad = np.zeros((E, E), np.float16)
    wq_pad[:KD] = Wq.astype(np.float16)
    tri_pad = np.zeros((E, KD), np.float16)
    tri_pad[:KD] = tri
    x16 = x.astype(np.float16)
    fp8_np = mybir.dt.np(FP8)
    in_maps = []
    for c in range(NCORES):
        b, h = divmod(c, 2)
        xb_ = x16[b]
        wpk = np.concatenate(
            [
                xb_[:KD].T,
                Wk.T.astype(np.float16),
                Wv.T.astype(np.float16),
                wq_pad,
                tri_pad,
            ],
            axis=1,
        )
        own_nat = xb_[h * HALF : (h + 1) * HALF]  # [2048, E] natural
        own = own_nat.T  # [E, 2048]
        other = xb_[(1 - h) * HALF : (2 - h) * HALF]  # [2048, E] natural
        # natural-tiled fp8: block j holds tokens j*128..j*128+127 on the
        # partition axis: pack[p, j*128+e] = src[j*128+p, e]
        def nat8(src):
            v = src.astype(fp8_np).reshape(16, TSUB, E).transpose(1, 0, 2)
            return np.ascontiguousarray(v).reshape(E, HALF)
        ot = nat8(other)
        in_maps.append(
            {
                "wpk": np.ascontiguousarray(wpk),
                "xowa": np.ascontiguousarray(own[:, 0:1024]),
                "xowb1": np.ascontiguousarray(own[:, 1024:1536]),
                "xowb2": np.ascontiguousarray(own[:, 1536:2048]),
                "xo8a": np.ascontiguousarray(ot[:, 0:1024]),
                "xo8b": np.ascontiguousarray(ot[:, 1024:2048]),
            }
        )
    return in_maps


def _gather(results):
    out = np.empty((B, S, E), np.float32)
    for c, r in enumerate(results):
        b, h = divmod(c, 2)
        # per-group device layout [p, t, v], token = (4g + t)*128 + p
        dev = np.concatenate(
            [np.asarray(r[f"o{g}"], dtype=np.float32) for g in range(4)], axis=1
        )
        out[b, h * HALF : (h + 1) * HALF] = dev.transpose(1, 0, 2).reshape(HALF, E)
    return out


def _run(x, Wk, Wq, Wv, **spmd_kwargs):
    nc = _get_nc()
    res = run_bass_kernel_spmd(
        nc,
        _make_in_maps(x, Wk, Wq, Wv),
        core_ids=list(range(NCORES)),
        **spmd_kwargs,
    )
    return _gather(res.results), res


def kernel(x, Wk, Wq, Wv):
    x = np.ascontiguousarray(np.asarray(x), dtype=np.float32)
    Wk = np.ascontiguousarray(np.asarray(Wk), dtype=np.float32)
    Wq = np.ascontiguousarray(np.asarray(Wq), dtype=np.float32)
    Wv = np.ascontiguousarray(np.asarray(Wv), dtype=np.float32)
    out, _ = _run(x, Wk, Wq, Wv)
    return out


# revision 36
# speedup vs baseline: 1.0044x; 1.0044x over previous
"""Masked self-attention Trainium2 kernel (v8 — queue-balanced, PE-summed tail).

Reference computes (per batch b):
    key   = x @ Wk.T            [S, 64]
    query = x @ Wq.T            [S, 64]
    value = x @ Wv.T            [S, 128]
    kT_m  = tril(key.T)         [64, S]   -- element (d, s) kept iff s <= d
    out   = softmax(query @ kT_m, axis=-1) @ value

tril zeroes every score column s >= 64, so with fixed shift c:

    out[t] = (sum_{s<64} e^{z_st-c} v[s] + e^{-c} Vtail) /
             (sum_{s<64} e^{z_st-c}      + e^{-c} (S-64))

with Vtail = (sum_{s>=64} x[s]) @ Wv.T.  Per core (batch b, half h):
z = Wz.T @ xow with Wz = tril-masked key64 folded into Wq (65th row = 0 so
pT row 64 = e^{-c}); out tiles are single K=65 matmuls against
vaug = [v64 rows | vtail row], den in the 129th column.

Trace-driven design notes (measured on HW, ~25us of which ~8.5us is the
fixed walrus sem-clear epilogue and ~2.5us fixed DMA startup):
- Queues (~150 GB/s per HWDGE queue, ~100 GB/s gpsimd SWDGE, ~0.9us
  DMA-completion-sem latency, ~0.7us per-queue DMA transition): sync:
  wpk -> xowb1; scalar: xowa -> xowb2; gpsimd: xo8a -> xo8b.  The own
  half is split 3 ways so z chunks and the own-half reduces unblock as
  their bytes land.
- Other half ships as fp8 natural-TILED 128-token blocks; its column sum
  runs on the PE as 16 tiny (block x fp8-ones) matmuls accumulating one
  PSUM column, so the scalar engine runs only the 4 exps plus scales.
- The tile scheduler orders each engine stream by ITS OWN sim; data-ready
  order on HW differs.  tile_wait_until hints (in sim-ms) pin the PE
  stream to [preamble, z0..z3, fp8 sums, vtail, tiles]; without them the
  sums (late fp8 data) park ahead of data-ready z matmuls and stall the
  in-order PE for microseconds.
- PSUM banks (8 x 2KB): z1/z2 ping-pong in zps; z0 + [osum | vtail |
  tile15] share one bank via sequential same-tag reuse; z3 takes the
  first oa-pool bank so its reuser (tiles 12-14) is exp3-gated anyway;
  tiles 0-14 pack 3-per-bank into the remaining 5 banks.  A PSUM bank
  tolerates only ONE open (start=True..stop) accumulation at a time --
  opening split-K accumulations in sibling slots corrupts them.
- Normalize: one strided [128,3] reciprocal per bank; 16 per-tile scales
  (PSUM->SBUF bf16, ~0.35/0.45us) split lane-pure across DVE and ACT so
  a store group never waits the other engine; stores alternate the sync
  and gpsimd queues.
"""

import numpy as np

import concourse.bass as bass
import concourse.bacc as bacc
import concourse.tile as tile
from concourse import mybir
from concourse.bass_utils import run_bass_kernel_spmd

F32 = mybir.dt.float32
F16 = mybir.dt.float16
BF16 = mybir.dt.bfloat16
FP8 = mybir.dt.float8e4
AF = mybir.ActivationFunctionType
AX = mybir.AxisListType
ALU = mybir.AluOpType

B, S, E, KD = 4, 4096, 128, 64
HALF = S // 2            # tokens handled per core
NCORES = 8
CHUNK = 512              # tokens per z-matmul / exp
NCHUNK = HALF // CHUNK
TSUB = 128               # tokens per output tile
NTILE = HALF // TSUB     # 16
CSHIFT = 20.0            # fixed softmax shift
NTAIL = float(S - KD)    # 4032 all-zero score columns
W = E + 1                # 129: num cols + den col per tile
NBANK = 6                # 3 tiles per PSUM bank (last bank holds 1)

# wpk columns: [x64T(64) | WkT(64) | Wq(128, rows 0:64) | tri(64, rows 0:64)]
X64_OFF, WK_OFF, WV_OFF, WQ_OFF, TRI_OFF = 0, KD, 2 * KD, 2 * KD + E, 2 * KD + 2 * E
WPK_COLS = 2 * KD + 2 * E + KD  # 448


def _build_nc() -> bass.Bass:
    nc = bacc.Bacc("TRN2", target_bir_lowering=False, debug=False)

    wpk = nc.dram_tensor("wpk", [E, WPK_COLS], F16, kind="ExternalInput").ap()
    xowa = nc.dram_tensor("xowa", [E, 1024], F16, kind="ExternalInput").ap()
    xowb1 = nc.dram_tensor("xowb1", [E, 512], F16, kind="ExternalInput").ap()
    xowb2 = nc.dram_tensor("xowb2", [E, 512], F16, kind="ExternalInput").ap()
    xo8a = nc.dram_tensor("xo8a", [E, 1024], FP8, kind="ExternalInput").ap()
    xo8b = nc.dram_tensor("xo8b", [E, 1024], FP8, kind="ExternalInput").ap()
    outs = [
        nc.dram_tensor(f"o{g}", [TSUB, 4, E], BF16, kind="ExternalOutput").ap()
        for g in range(4)
    ]

    with tile.TileContext(nc) as tc:
        with (
            tc.tile_pool(name="singles", bufs=1) as singles,
            tc.tile_pool(name="zps", bufs=2, space="PSUM") as zps,
            tc.tile_pool(name="misc_ps", bufs=1, space="PSUM") as misc_ps,
            tc.tile_pool(name="oa_ps", bufs=5, space="PSUM") as oa_ps,
            tc.tile_pool(name="recs", bufs=6) as recs,
            tc.tile_pool(name="obs", bufs=4) as obs,
        ):
            # ---- DMA in (queue order == issue order per engine) ----
            wpk_sb = singles.tile([E, WPK_COLS], F16)
            nc.sync.dma_start(wpk_sb[:], wpk)
            xow_sb = singles.tile([E, HALF], F16)
            xo8_sb = singles.tile([E, HALF], FP8)
            nc.sync.dma_start(xow_sb[:, 1024:1536], xowb1)
            nc.scalar.dma_start(xow_sb[:, 0:1024], xowa)
            nc.scalar.dma_start(xow_sb[:, 1536:2048], xowb2)
            nc.gpsimd.dma_start(xo8_sb[:, 0:1024], xo8a)
            nc.gpsimd.dma_start(xo8_sb[:, 1024:2048], xo8b)

            x64T_sb = wpk_sb[:, X64_OFF : X64_OFF + KD]
            wkT_sb = wpk_sb[:, WK_OFF : WK_OFF + KD]
            wvT_sb = wpk_sb[:, WV_OFF : WV_OFF + E]
            wq_sb = wpk_sb[0:KD, WQ_OFF : WQ_OFF + E]
            tri_sb = wpk_sb[0:KD, TRI_OFF : TRI_OFF + KD]

            # ---- constants (gpsimd is otherwise idle early) ----
            wzaug_sb = singles.tile([E, KD + 1], F16)
            nc.gpsimd.memset(wzaug_sb[:, KD : KD + 1], 0.0)
            vaug_sb = singles.tile([KD + 1, W], BF16)
            nc.gpsimd.memset(vaug_sb[0:KD, E : E + 1], 1.0)
            nc.gpsimd.memset(vaug_sb[KD : KD + 1, E : E + 1], NTAIL)
            nbias_sb = singles.tile([KD + 1, 1], F32)
            nc.gpsimd.memset(nbias_sb[:], -CSHIFT)
            ones8_sb = singles.tile([E, 1], FP8)
            nc.gpsimd.memset(ones8_sb[:], 1.0)

            # ---- preamble: build Wz (score weights) and v64 ----
            with tc.high_priority():
                kT_ps = zps.tile([KD, KD], F32, tag="z", name="kT_ps")
                nc.tensor.matmul(kT_ps[:], wkT_sb, x64T_sb, start=True, stop=True)
                kmT_sb = singles.tile([KD, KD], F16)
                nc.vector.tensor_mul(kmT_sb[:], kT_ps[:], tri_sb)
                wzT_ps = zps.tile([E, KD], F32, tag="z", name="wzT_ps")
                nc.tensor.matmul(wzT_ps[:], wq_sb, kmT_sb[:], start=True, stop=True)
                nc.vector.tensor_copy(wzaug_sb[:, 0:KD], wzT_ps[:])
                v64_ps = zps.tile([KD, E], F32, tag="z", name="v64_ps")
                nc.tensor.matmul(v64_ps[:], x64T_sb, wvT_sb, start=True, stop=True)
                nc.scalar.activation(vaug_sb[0:KD, 0:E], v64_ps[:], AF.Copy)

            x64s_sb = singles.tile([E, 1], F32)
            nc.vector.reduce_sum(out=x64s_sb[:], in_=x64T_sb, axis=AX.X)

            # ---- z chunks + exps; fp8 PE-sums woven into PE gaps ----
            pT_sb = singles.tile([KD + 1, HALF], BF16)

            def zexp(c, w):
                cs = slice(c * CHUNK, (c + 1) * CHUNK)
                if c == 0:
                    z_ps = misc_ps.tile([KD + 1, CHUNK], F32, tag="m", name="z0_ps")
                elif c == 3:
                    # fresh bank from the oa pool: its eventual reuser
                    # (tiles 12-14) is gated on exp3 anyway
                    z_ps = oa_ps.tile([KD + 1, CHUNK], F32, tag="oa", name="z3_ps")
                else:
                    z_ps = zps.tile([KD + 1, CHUNK], F32, tag="z", name=f"z{c}_ps")
                with tc.tile_wait_until(w):
                    nc.tensor.matmul(
                        z_ps[:], wzaug_sb[:], xow_sb[:, cs], start=True, stop=True
                    )
                    nc.scalar.activation(
                        pT_sb[0 : KD + 1, cs], z_ps[:], AF.Exp, bias=nbias_sb[:]
                    )

            def osums(src, half, first, last):
                for j in range(8):
                    js = slice(half * 1024 + j * TSUB, half * 1024 + (j + 1) * TSUB)
                    nc.tensor.matmul(
                        osum_ps, src[:, js], ones8_sb[:],
                        start=(first and j == 0), stop=(last and j == 7),
                    )

            zexp(0, 0.0005)
            zexp(1, 0.001)
            zexp(2, 0.0015)
            zexp(3, 0.002)
            # misc pack shares the z0 bank (sequential same-tag reuse):
            # col 0 = osum, row0 cols 1:130 = vtail, cols 130:259 = tile 15
            mpk = misc_ps.tile([E, 263], F32, tag="m", name="miscpack_ps")
            osum_ps = mpk[:, 0:1]
            with tc.tile_wait_until(0.005):
                osums(xo8_sb, 0, True, False)
                osums(xo8_sb, 1, False, True)

            # ---- own-half sums (DVE) and the tail vector ----
            rdA_sb = singles.tile([E, 1], F32)
            rdB_sb = singles.tile([E, 1], F32)
            rdC_sb = singles.tile([E, 1], F32)
            with tc.tile_wait_until(0.003):
                nc.vector.reduce_sum(out=rdA_sb[:], in_=xow_sb[:, 0:1024], axis=AX.X)
            with tc.tile_wait_until(0.004):
                nc.vector.reduce_sum(out=rdC_sb[:], in_=xow_sb[:, 1024:1536], axis=AX.X)
            with tc.tile_wait_until(0.0045):
                nc.vector.reduce_sum(out=rdB_sb[:], in_=xow_sb[:, 1536:2048], axis=AX.X)
            w_sb = singles.tile([E, 1], F32)
            nc.vector.tensor_sub(w_sb[:], x64s_sb[:], rdB_sb[:])
            u1_sb = singles.tile([E, 1], F32)
            nc.vector.scalar_tensor_tensor(
                u1_sb[:], rdA_sb[:], rdC_sb[:], osum_ps, ALU.add, ALU.add
            )
            tailh_sb = singles.tile([E, 1], F16)
            nc.vector.tensor_sub(tailh_sb[:], u1_sb[:], w_sb[:])

            vtail_ps = mpk[0:1, 1:129]
            nc.tensor.matmul(vtail_ps, tailh_sb[:], wvT_sb, start=True, stop=True)
            with tc.tile_wait_until(0.0055):
                nc.scalar.activation(vaug_sb[KD : KD + 1, 0:E], vtail_ps, AF.Copy)

            # ---- output tiles: 3 per PSUM bank, K=65 single matmuls ----
            # t0-14 fill the 5 oa banks exactly (3 per bank); t15 lives in
            # the misc pack (no 6th bank allocation / WAR stall)
            oa_banks = []
            slot_of = {}

            def slot_ap(t):
                if t == 15:
                    return mpk[:, 130 : 130 + W]
                b, j = slot_of[t]
                return oa_banks[b][:, j * W : j * W + W]

            for t in range(15):
                if t % 3 == 0:
                    oa_banks.append(
                        oa_ps.tile([TSUB, 3 * W], F32, tag="oa", name=f"oa{t // 3}")
                    )
                slot_of[t] = (t // 3, t % 3)
            for t in range(NTILE):
                ts = slice(t * TSUB, (t + 1) * TSUB)
                nc.tensor.matmul(
                    slot_ap(t), pT_sb[0 : KD + 1, ts], vaug_sb[:],
                    start=True, stop=True,
                )

            # ---- normalize + store ----
            rec_tiles = []
            for b in range(5):
                rec = recs.tile([TSUB, 3], F32, tag="rec", name=f"rec{b}")
                rec_tiles.append(rec)
                nc.vector.reciprocal(rec[:, 0:3], oa_banks[b][:, E :: W][:, 0:3])
            rec15 = recs.tile([TSUB, 1], F32, tag="rec15", name="rec15")
            nc.vector.reciprocal(rec15[:], mpk[:, 130 + E : 130 + E + 1])

            ob_tiles = [
                obs.tile([TSUB, 4, E], BF16, tag="ob", name=f"ob{g}")
                for g in range(4)
            ]
            ndone = [0] * 4
            for t in range(NTILE):
                g, gj = divmod(t, 4)
                ob = ob_tiles[g]
                oa = slot_ap(t)
                if t == 15:
                    rec = rec15[:]
                else:
                    b, j = slot_of[t]
                    rec = rec_tiles[b][:, j : j + 1]
                # lane-pure store groups: g0/g3 all-DVE, g1/g2 all-ACT
                if g in (0, 3):
                    nc.vector.tensor_scalar_mul(ob[:, gj, :], oa[:, 0:E], rec)
                else:
                    nc.scalar.activation(ob[:, gj, :], oa[:, 0:E], AF.Copy, scale=rec)
                ndone[g] += 1
                if ndone[g] == 4:
                    eng = (nc.gpsimd, nc.gpsimd, nc.sync, nc.sync)[g]
                    eng.dma_start(outs[g], ob[:])

    nc.compile()
    return nc


_NC_CACHE = None


def _get_nc() -> bass.Bass:
    global _NC_CACHE
    if _NC_CACHE is None:
        _NC_CACHE = _build_nc()
    return _NC_CACHE


def _make_in_maps(x, Wk, Wq, Wv):
    tri = (np.arange(KD)[:, None] >= np.arange(KD)[None, :]).astype(np.float16)
    wq_pad = np.zeros((E, E), np.float16)
    wq_pad[:KD] = Wq.astype(np.float16)
    tri_pad = np.zeros((E, KD), np.float16)
    tri_pad[:KD] = tri
    x16 = x.astype(np.float16)
    fp8_np = mybir.dt.np(FP8)
    in_maps = []
    for c in range(NCORES):
        b, h = divmod(c, 2)
        xb_ = x16[b]
        wpk = np.concatenate(
            [
                xb_[:KD].T,
                Wk.T.astype(np.float16),
                Wv.T.astype(np.float16),
                wq_pad,
                tri_pad,
            ],
            axis=1,
        )
        own_nat = xb_[h * HALF : (h + 1) * HALF]  # [2048, E] natural
        own = own_nat.T  # [E, 2048]
        other = xb_[(1 - h) * HALF : (2 - h) * HALF]  # [2048, E] natural
        # natural-tiled fp8: block j holds tokens j*128..j*128+127 on the
        # partition axis: pack[p, j*128+e] = src[j*128+p, e]
        def nat8(src):
            v = src.astype(fp8_np).reshape(16, TSUB, E).transpose(1, 0, 2)
            return np.ascontiguousarray(v).reshape(E, HALF)
        ot = nat8(other)
        in_maps.append(
            {
                "wpk": np.ascontiguousarray(wpk),
                "xowa": np.ascontiguousarray(own[:, 0:1024]),
                "xowb1": np.ascontiguousarray(own[:, 1024:1536]),
                "xowb2": np.ascontiguousarray(own[:, 1536:2048]),
                "xo8a": np.ascontiguousarray(ot[:, 0:1024]),
                "xo8b": np.ascontiguousarray(ot[:, 1024:2048]),
            }
        )
    return in_maps


def _gather(results):
    out = np.empty((B, S, E), np.float32)
    for c, r in enumerate(results):
        b, h = divmod(c, 2)
        # per-group device layout [p, t, v], token = (4g + t)*128 + p
        dev = np.concatenate(
            [np.asarray(r[f"o{g}"], dtype=np.float32) for g in range(4)], axis=1
        )
        out[b, h * HALF : (h + 1) * HALF] = dev.transpose(1, 0, 2).reshape(HALF, E)
    return out


def _run(x, Wk, Wq, Wv, **spmd_kwargs):
    nc = _get_nc()
    res = run_bass_kernel_spmd(
        nc,
        _make_in_maps(x, Wk, Wq, Wv),
        core_ids=list(range(NCORES)),
        **spmd_kwargs,
    )
    return _gather(res.results), res


def kernel(x, Wk, Wq, Wv):
    x = np.ascontiguousarray(np.asarray(x), dtype=np.float32)
    Wk = np.ascontiguousarray(np.asarray(Wk), dtype=np.float32)
    Wq = np.ascontiguousarray(np.asarray(Wq), dtype=np.float32)
    Wv = np.ascontiguousarray(np.asarray(Wv), dtype=np.float32)
    out, _ = _run(x, Wk, Wq, Wv)
    return out


# revision 37
# speedup vs baseline: 1.0299x; 1.0254x over previous
"""Masked self-attention Trainium2 kernel (v8 — queue-balanced, PE-summed tail).

Reference computes (per batch b):
    key   = x @ Wk.T            [S, 64]
    query = x @ Wq.T            [S, 64]
    value = x @ Wv.T            [S, 128]
    kT_m  = tril(key.T)         [64, S]   -- element (d, s) kept iff s <= d
    out   = softmax(query @ kT_m, axis=-1) @ value

tril zeroes every score column s >= 64, so with fixed shift c:

    out[t] = (sum_{s<64} e^{z_st-c} v[s] + e^{-c} Vtail) /
             (sum_{s<64} e^{z_st-c}      + e^{-c} (S-64))

with Vtail = (sum_{s>=64} x[s]) @ Wv.T.  Per core (batch b, half h):
z = Wz.T @ xow with Wz = tril-masked key64 folded into Wq (65th row = 0 so
pT row 64 = e^{-c}); out tiles are single K=65 matmuls against
vaug = [v64 rows | vtail row], den in the 129th column.

Trace-driven design notes (measured on HW, ~25us of which ~8.5us is the
fixed walrus sem-clear epilogue and ~2.5us fixed DMA startup):
- Queues (~150 GB/s per HWDGE queue, ~100 GB/s gpsimd SWDGE, ~0.9us
  DMA-completion-sem latency, ~0.7us per-queue DMA transition): sync:
  wpk -> xowb1; scalar: xowa -> xowb2; gpsimd: xo8a -> xo8b.  The own
  half is split 3 ways so z chunks and the own-half reduces unblock as
  their bytes land.
- Other half ships as fp8 natural-TILED 128-token blocks; its column sum
  runs on the PE as 16 tiny (block x fp8-ones) matmuls accumulating one
  PSUM column, so the scalar engine runs only the 4 exps plus scales.
- The tile scheduler orders each engine stream by ITS OWN sim; data-ready
  order on HW differs.  tile_wait_until hints (in sim-ms) pin the PE
  stream to [preamble, z0..z3, fp8 sums, vtail, tiles]; without them the
  sums (late fp8 data) park ahead of data-ready z matmuls and stall the
  in-order PE for microseconds.
- PSUM banks (8 x 2KB): z1/z2 ping-pong in zps; z0 + [osum | vtail |
  tile15] share one bank via sequential same-tag reuse; z3 takes the
  first oa-pool bank so its reuser (tiles 12-14) is exp3-gated anyway;
  tiles 0-14 pack 3-per-bank into the remaining 5 banks.  A PSUM bank
  tolerates only ONE open (start=True..stop) accumulation at a time --
  opening split-K accumulations in sibling slots corrupts them.
- Normalize: one strided [128,3] reciprocal per bank; 16 per-tile scales
  (PSUM->SBUF bf16, ~0.35/0.45us) split lane-pure across DVE and ACT so
  a store group never waits the other engine; stores alternate the sync
  and gpsimd queues.
"""

import numpy as np

import concourse.bass as bass
import concourse.bacc as bacc
import concourse.tile as tile
from concourse import mybir
from concourse.bass_utils import run_bass_kernel_spmd

F32 = mybir.dt.float32
F16 = mybir.dt.float16
BF16 = mybir.dt.bfloat16
FP8 = mybir.dt.float8e4
AF = mybir.ActivationFunctionType
AX = mybir.AxisListType
ALU = mybir.AluOpType

B, S, E, KD = 4, 4096, 128, 64
HALF = S // 2            # tokens handled per core
NCORES = 8
CHUNK = 512              # tokens per z-matmul / exp
NCHUNK = HALF // CHUNK
TSUB = 128               # tokens per output tile
NTILE = HALF // TSUB     # 16
CSHIFT = 20.0            # fixed softmax shift
NTAIL = float(S - KD)    # 4032 all-zero score columns
W = E + 1                # 129: num cols + den col per tile
NBANK = 6                # 3 tiles per PSUM bank (last bank holds 1)

# wpk columns: [x64T(64) | WkT(64) | Wq(128, rows 0:64) | tri(64, rows 0:64)]
X64_OFF, WK_OFF, WV_OFF, WQ_OFF, TRI_OFF = 0, KD, 2 * KD, 2 * KD + E, 2 * KD + 2 * E
WPK_COLS = 2 * KD + 2 * E + KD  # 448


def _build_nc() -> bass.Bass:
    nc = bacc.Bacc("TRN2", target_bir_lowering=False, debug=False)

    wpk = nc.dram_tensor("wpk", [E, WPK_COLS], F16, kind="ExternalInput").ap()
    xowa = nc.dram_tensor("xowa", [E, 1024], F16, kind="ExternalInput").ap()
    xowb1 = nc.dram_tensor("xowb1", [E, 512], F16, kind="ExternalInput").ap()
    xowb2 = nc.dram_tensor("xowb2", [E, 512], F16, kind="ExternalInput").ap()
    xo8a = nc.dram_tensor("xo8a", [E, 1024], FP8, kind="ExternalInput").ap()
    xo8b = nc.dram_tensor("xo8b", [E, 1024], FP8, kind="ExternalInput").ap()
    outs = [
        nc.dram_tensor(f"o{g}", [TSUB, 4, E], BF16, kind="ExternalOutput").ap()
        for g in range(4)
    ]

    with tile.TileContext(nc) as tc:
        with (
            tc.tile_pool(name="singles", bufs=1) as singles,
            tc.tile_pool(name="zps", bufs=2, space="PSUM") as zps,
            tc.tile_pool(name="misc_ps", bufs=1, space="PSUM") as misc_ps,
            tc.tile_pool(name="oa_ps", bufs=5, space="PSUM") as oa_ps,
            tc.tile_pool(name="recs", bufs=6) as recs,
            tc.tile_pool(name="obs", bufs=4) as obs,
        ):
            # ---- DMA in (queue order == issue order per engine) ----
            wpk_sb = singles.tile([E, WPK_COLS], F16)
            nc.sync.dma_start(wpk_sb[:], wpk)
            xow_sb = singles.tile([E, HALF], F16)
            xo8_sb = singles.tile([E, HALF], FP8)
            nc.sync.dma_start(xow_sb[:, 1024:1536], xowb1)
            nc.scalar.dma_start(xow_sb[:, 0:1024], xowa)
            nc.scalar.dma_start(xow_sb[:, 1536:2048], xowb2)
            nc.gpsimd.dma_start(xo8_sb[:, 0:1024], xo8a)
            nc.gpsimd.dma_start(xo8_sb[:, 1024:2048], xo8b)

            x64T_sb = wpk_sb[:, X64_OFF : X64_OFF + KD]
            wkT_sb = wpk_sb[:, WK_OFF : WK_OFF + KD]
            wvT_sb = wpk_sb[:, WV_OFF : WV_OFF + E]
            wq_sb = wpk_sb[0:KD, WQ_OFF : WQ_OFF + E]
            tri_sb = wpk_sb[0:KD, TRI_OFF : TRI_OFF + KD]

            # ---- constants (gpsimd is otherwise idle early) ----
            wzaug_sb = singles.tile([E, KD + 1], F16)
            nc.gpsimd.memset(wzaug_sb[:, KD : KD + 1], 0.0)
            vaug_sb = singles.tile([KD + 1, W], BF16)
            nc.gpsimd.memset(vaug_sb[0:KD, E : E + 1], 1.0)
            nc.gpsimd.memset(vaug_sb[KD : KD + 1, E : E + 1], NTAIL)
            nbias_sb = singles.tile([KD + 1, 1], F32)
            nc.gpsimd.memset(nbias_sb[:], -CSHIFT)
            ones8_sb = singles.tile([E, 1], FP8)
            nc.gpsimd.memset(ones8_sb[:], 1.0)

            # ---- preamble: build Wz (score weights) and v64 ----
            with tc.high_priority():
                kT_ps = zps.tile([KD, KD], F32, tag="z", name="kT_ps")
                nc.tensor.matmul(kT_ps[:], wkT_sb, x64T_sb, start=True, stop=True)
                kmT_sb = singles.tile([KD, KD], F16)
                nc.vector.tensor_mul(kmT_sb[:], kT_ps[:], tri_sb)
                wzT_ps = zps.tile([E, KD], F32, tag="z", name="wzT_ps")
                nc.tensor.matmul(wzT_ps[:], wq_sb, kmT_sb[:], start=True, stop=True)
                nc.vector.tensor_copy(wzaug_sb[:, 0:KD], wzT_ps[:])
                v64_ps = zps.tile([KD, E], F32, tag="z", name="v64_ps")
                nc.tensor.matmul(v64_ps[:], x64T_sb, wvT_sb, start=True, stop=True)
                nc.scalar.activation(vaug_sb[0:KD, 0:E], v64_ps[:], AF.Copy)

            x64s_sb = singles.tile([E, 1], F32)
            nc.vector.reduce_sum(out=x64s_sb[:], in_=x64T_sb, axis=AX.X)

            # ---- z chunks + exps; fp8 PE-sums woven into PE gaps ----
            pT_sb = singles.tile([KD + 1, HALF], BF16)

            def zexp(c, w):
                cs = slice(c * CHUNK, (c + 1) * CHUNK)
                if c == 0:
                    z_ps = misc_ps.tile([KD + 1, CHUNK], F32, tag="m", name="z0_ps")
                elif c == 3:
                    # fresh bank from the oa pool: its eventual reuser
                    # (tiles 12-14) is gated on exp3 anyway
                    z_ps = oa_ps.tile([KD + 1, CHUNK], F32, tag="oa", name="z3_ps")
                else:
                    z_ps = zps.tile([KD + 1, CHUNK], F32, tag="z", name=f"z{c}_ps")
                with tc.tile_wait_until(w):
                    nc.tensor.matmul(
                        z_ps[:], wzaug_sb[:], xow_sb[:, cs], start=True, stop=True
                    )
                    nc.scalar.activation(
                        pT_sb[0 : KD + 1, cs], z_ps[:], AF.Exp, bias=nbias_sb[:]
                    )

            def osums(src, half, first, last):
                for j in range(8):
                    js = slice(half * 1024 + j * TSUB, half * 1024 + (j + 1) * TSUB)
                    nc.tensor.matmul(
                        osum_ps, src[:, js], ones8_sb[:],
                        start=(first and j == 0), stop=(last and j == 7),
                    )

            zexp(0, 0.0005)
            zexp(1, 0.001)
            zexp(2, 0.0015)
            zexp(3, 0.002)
            # misc pack shares the z0 bank (sequential same-tag reuse):
            # col 0 = osum, row0 cols 1:130 = vtail, cols 130:259 = tile 15
            mpk = misc_ps.tile([E, 263], F32, tag="m", name="miscpack_ps")
            osum_ps = mpk[:, 0:1]
            with tc.tile_wait_until(0.005):
                osums(xo8_sb, 0, True, False)
                osums(xo8_sb, 1, False, True)

            # ---- own-half sums (DVE) and the tail vector ----
            rdA_sb = singles.tile([E, 1], F32)
            rdB_sb = singles.tile([E, 1], F32)
            rdC_sb = singles.tile([E, 1], F32)
            with tc.tile_wait_until(0.003):
                nc.vector.reduce_sum(out=rdA_sb[:], in_=xow_sb[:, 0:1024], axis=AX.X)
            with tc.tile_wait_until(0.004):
                nc.vector.reduce_sum(out=rdC_sb[:], in_=xow_sb[:, 1024:1536], axis=AX.X)
            with tc.tile_wait_until(0.0045):
                nc.vector.reduce_sum(out=rdB_sb[:], in_=xow_sb[:, 1536:2048], axis=AX.X)
            w_sb = singles.tile([E, 1], F32)
            nc.vector.tensor_sub(w_sb[:], x64s_sb[:], rdB_sb[:])
            u1_sb = singles.tile([E, 1], F32)
            nc.vector.scalar_tensor_tensor(
                u1_sb[:], rdA_sb[:], rdC_sb[:], osum_ps, ALU.add, ALU.add
            )
            tailh_sb = singles.tile([E, 1], F16)
            nc.vector.tensor_sub(tailh_sb[:], u1_sb[:], w_sb[:])

            vtail_ps = mpk[0:1, 1:129]
            nc.tensor.matmul(vtail_ps, tailh_sb[:], wvT_sb, start=True, stop=True)
            with tc.tile_wait_until(0.0055):
                nc.scalar.activation(vaug_sb[KD : KD + 1, 0:E], vtail_ps, AF.Copy)

            # ---- output tiles: 3 per PSUM bank, K=65 single matmuls ----
            # t0-14 fill the 5 oa banks exactly (3 per bank); t15 lives in
            # the misc pack (no 6th bank allocation / WAR stall)
            oa_banks = []
            slot_of = {}

            def slot_ap(t):
                if t == 15:
                    return mpk[:, 130 : 130 + W]
                b, j = slot_of[t]
                return oa_banks[b][:, j * W : j * W + W]

            for t in range(15):
                if t % 3 == 0:
                    oa_banks.append(
                        oa_ps.tile([TSUB, 3 * W], F32, tag="oa", name=f"oa{t // 3}")
                    )
                slot_of[t] = (t // 3, t % 3)
            for t in range(NTILE):
                ts = slice(t * TSUB, (t + 1) * TSUB)
                nc.tensor.matmul(
                    slot_ap(t), pT_sb[0 : KD + 1, ts], vaug_sb[:],
                    start=True, stop=True,
                )

            # ---- normalize + store ----
            rec_tiles = []
            for b in range(5):
                rec = recs.tile([TSUB, 3], F32, tag="rec", name=f"rec{b}")
                rec_tiles.append(rec)
                nc.vector.reciprocal(rec[:, 0:3], oa_banks[b][:, E :: W][:, 0:3])
            rec15 = recs.tile([TSUB, 1], F32, tag="rec15", name="rec15")
            nc.vector.reciprocal(rec15[:], mpk[:, 130 + E : 130 + E + 1])

            ob_tiles = [
                obs.tile([TSUB, 4, E], BF16, tag="ob", name=f"ob{g}")
                for g in range(4)
            ]
            ndone = [0] * 4
            for t in range(NTILE):
                g, gj = divmod(t, 4)
                ob = ob_tiles[g]
                oa = slot_ap(t)
                if t == 15:
                    rec = rec15[:]
                else:
                    b, j = slot_of[t]
                    rec = rec_tiles[b][:, j : j + 1]
                # lane-pure store groups: g0/g3 all-DVE, g1/g2 all-ACT
                if g in (0, 3):
                    nc.vector.tensor_scalar_mul(ob[:, gj, :], oa[:, 0:E], rec)
                else:
                    nc.scalar.activation(ob[:, gj, :], oa[:, 0:E], AF.Copy, scale=rec)
                ndone[g] += 1
                if ndone[g] == 4:
                    eng = (nc.sync, nc.gpsimd, nc.sync, nc.gpsimd)[g]
                    eng.dma_start(outs[g], ob[:])

    nc.compile()
    return nc


_NC_CACHE = None


def _get_nc() -> bass.Bass:
    global _NC_CACHE
    if _NC_CACHE is None:
        _NC_CACHE = _build_nc()
    return _NC_CACHE


def _make_in_maps(x, Wk, Wq, Wv):
    tri = (np.arange(KD)[:, None] >= np.arange(KD)[None, :]).astype(np.float16)
    wq_pad = np.zeros((E, E), np.float16)
    wq_pad[:KD] = Wq.astype(np.float16)
    tri_pad = np.zeros((E, KD), np.float16)
    tri_pad[:KD] = tri
    x16 = x.astype(np.float16)
    fp8_np = mybir.dt.np(FP8)
    in_maps = []
    for c in range(NCORES):
        b, h = divmod(c, 2)
        xb_ = x16[b]
        wpk = np.concatenate(
            [
                xb_[:KD].T,
                Wk.T.astype(np.float16),
                Wv.T.astype(np.float16),
                wq_pad,
                tri_pad,
            ],
            axis=1,
        )
        own_nat = xb_[h * HALF : (h + 1) * HALF]  # [2048, E] natural
        own = own_nat.T  # [E, 2048]
        other = xb_[(1 - h) * HALF : (2 - h) * HALF]  # [2048, E] natural
        # natural-tiled fp8: block j holds tokens j*128..j*128+127 on the
        # partition axis: pack[p, j*128+e] = src[j*128+p, e]
        def nat8(src):
            v = src.astype(fp8_np).reshape(16, TSUB, E).transpose(1, 0, 2)
            return np.ascontiguousarray(v).reshape(E, HALF)
        ot = nat8(other)
        in_maps.append(
            {
                "wpk": np.ascontiguousarray(wpk),
                "xowa": np.ascontiguousarray(own[:, 0:1024]),
                "xowb1": np.ascontiguousarray(own[:, 1024:1536]),
                "xowb2": np.ascontiguousarray(own[:, 1536:2048]),
                "xo8a": np.ascontiguousarray(ot[:, 0:1024]),
                "xo8b": np.ascontiguousarray(ot[:, 1024:2048]),
            }
        )
    return in_maps


def _gather(results):
    out = np.empty((B, S, E), np.float32)
    for c, r in enumerate(results):
        b, h = divmod(c, 2)
        # per-group device layout [p, t, v], token = (4g + t)*128 + p
        dev = np.concatenate(
            [np.asarray(r[f"o{g}"], dtype=np.float32) for g in range(4)], axis=1
        )
        out[b, h * HALF : (h + 1) * HALF] = dev.transpose(1, 0, 2).reshape(HALF, E)
    return out


def _run(x, Wk, Wq, Wv, **spmd_kwargs):
    nc = _get_nc()
    res = run_bass_kernel_spmd(
        nc,
        _make_in_maps(x, Wk, Wq, Wv),
        core_ids=list(range(NCORES)),
        **spmd_kwargs,
    )
    return _gather(res.results), res


def kernel(x, Wk, Wq, Wv):
    x = np.ascontiguousarray(np.asarray(x), dtype=np.float32)
    Wk = np.ascontiguousarray(np.asarray(Wk), dtype=np.float32)
    Wq = np.ascontiguousarray(np.asarray(Wq), dtype=np.float32)
    Wv = np.ascontiguousarray(np.asarray(Wv), dtype=np.float32)
    out, _ = _run(x, Wk, Wq, Wv)
    return out
